# revision 91
# baseline (speedup 1.0000x reference)
"""CPWanSelfAttention on 8 Trainium2 NeuronCores.

Strategy: tensor-parallel over heads (16 heads -> 2 per core).
Per core c (heads 2c, 2c+1):
  - qT/kT = wq_c @ hiddenT in transposed-per-head layout [dhead, S] (bf16
    inputs), head dims host-permuted to [evens..., odds...] so RoPE pair
    math becomes aligned half-tile ops.
  - v in natural [S, d] layout (PV stationary operand).
  - RMS norm: per-core partial sum-of-squares via a second ACT Square drain
    of the QKV psum -> ONE merged AllReduce [1, 2*S] -> rstd via
    partition-broadcast DMA + Sqrt + reciprocal; RoPE fused with the norm
    scaling (scalar_tensor_tensor) on DVE, f32 intermediates, f32r q/k.
  - scoresT[k, q] = k @ qT per head in f32r; exp on ScalarE with a constant
    -4 logit shift (numerator and denominator scale identically); the
    denominator accumulates on PE via an all-ones [128,128] stationary so it
    lands pre-broadcast; divide tail (recip+mul) on DVE.
  - Per-head outputs oT (bf16) are re-sharded by TOKEN via four AllToAlls
    (one per seq half x head, issued as each head finishes so they pipeline
    on the collective engine): core m ends up with all 16 heads' o for its
    token blocks {m*128..} and {1024+m*128..}. Each core then computes the
    FULL 2048-dim output projection for its 256 tokens against streamed
    bf16 wo column-groups, contraction split by head parity so the even half
    starts before the last AllToAll lands. Host re-interleaves token blocks.
    (vs AllGather of o: 8x less collective traffic, no 16MB regather DMA.)
Engine placement is deliberate: ropes on DVE, v-bias adds + collectives on
GpSimd, strip/och/outTok DMAs on SP, sumsq drains on ACT - keeping each
in-order queue free of cross-phase stalls. Attention-phase SBUF pools are
allocated at top scope so they never alias phase-1 buffers (aliasing makes
the first attention tiles wait for the rope tail).
"""

from contextlib import ExitStack

import numpy as np
import concourse.bass as bass
import concourse.mybir as mybir
import concourse.tile as tile
from concourse import bacc
from concourse.bass_utils import run_bass_kernel_spmd

N_CORES = 8
S = 1992
SP = 2048          # seq padded to multiple of 128 (nki flash attention contract)
DIM = 2048
NHEADS = 16
DH = 128
HPC = NHEADS // N_CORES   # heads per core = 2
DC = DH * HPC             # out dims per core = 256
KT = DIM // 128           # 16 contraction tiles
NCH = SP // 512           # 4 seq chunks of 512
EPS = 1e-6

F32 = mybir.dt.float32
F32R = mybir.dt.float32r
BF16 = mybir.dt.bfloat16
F8 = mybir.dt.float8e4
DR = mybir.MatmulPerfMode.DoubleRow
WSCALE = 1.0       # no prescale needed at bf16

_COMPILED = None
DEBUG_DUMP = None


def _build(ag_mode='ato2', repeat=1, stage=4):
    nc = bacc.Bacc("TRN2", target_bir_lowering=False, debug=False,
                   num_devices=N_CORES)

    # ---- DRAM I/O (per-core shards; float32r tensors feed matmuls) ----
    hidT = nc.dram_tensor("hidT", [SP // 512, 128, KT, 512], BF16, kind="ExternalInput")
    wqT = nc.dram_tensor("wqT", [128, KT, DC], BF16, kind="ExternalInput")
    wkT = nc.dram_tensor("wkT", [128, KT, DC], BF16, kind="ExternalInput")
    wvT = nc.dram_tensor("wvT", [128, KT, DC], BF16, kind="ExternalInput")
    woT = nc.dram_tensor("woT", [128, KT, DIM], BF16, kind="ExternalInput")
    cosT = nc.dram_tensor("cosT", [DH, SP], F32, kind="ExternalInput")  # [c;c]
    sinT = nc.dram_tensor("sinT", [DH, SP], F32, kind="ExternalInput")  # [-s;s]
    bq = nc.dram_tensor("bq", [HPC, DH], F32, kind="ExternalInput")
    bk = nc.dram_tensor("bk", [HPC, DH], F32, kind="ExternalInput")
    bv = nc.dram_tensor("bv", [1, DC], F32, kind="ExternalInput")
    bo = nc.dram_tensor("bo", [1, DIM], F32, kind="ExternalInput")
    nwq = nc.dram_tensor("nwq", [HPC, DH], F32, kind="ExternalInput")
    nwk = nc.dram_tensor("nwk", [HPC, DH], F32, kind="ExternalInput")
    ones8 = nc.dram_tensor("ones8", [128, 128], BF16, kind="ExternalInput")
    # per-core output: 2 token blocks of 128 (one per seq half) x full dim
    outTok = nc.dram_tensor("outTok", [2, 128, DIM], F32, kind="ExternalOutput")
    dbg = None
    if DEBUG_DUMP in ("qraw", "kTt", "oT", "qT"):
        dbg = nc.dram_tensor("dbg", [128, SP], BF16, kind="ExternalOutput")
    elif DEBUG_DUMP == "vsb":
        dbg = nc.dram_tensor("dbg", [128, SP // 128 * DC], F8,
                             kind="ExternalOutput")

    rg = [list(range(N_CORES))]
    inv_sqrt_dh = 1.0 / float(np.sqrt(DH))

    def emit(tc, top, rep):
        P = lambda nm: f"{nm}_{rep}"
        const = top.enter_context(tc.tile_pool(name=P("const"), bufs=1))
        pv_pool = top.enter_context(tc.tile_pool(name=P("pv_pool"), bufs=1))
        dram = top.enter_context(tc.tile_pool(name=P("dram"), bufs=1, space="DRAM"))

        ones_col = const.tile([128, 1], BF16)
        nc.vector.memset(ones_col[:], 1.0)
        # [128,128] of ones: the denominator matmul then yields the softmax
        # denominator already broadcast across all partitions
        ones_den = const.tile([128, 128], BF16)
        nc.sync.dma_start(ones_den[:], ones8[:])
        ones_1row = const.tile([1, 128], F32R)
        nc.vector.memset(ones_1row[:].bitcast(F32), 1.0)
        bq_sb = const.tile([128, HPC], F32)
        bk_sb = const.tile([128, HPC], F32)
        nwq_sb = const.tile([128, HPC], F32)
        nwk_sb = const.tile([128, HPC], F32)
        nc.sync.dma_start(bq_sb[:], bq[:].rearrange("h p -> p h"))
        nc.sync.dma_start(bk_sb[:], bk[:].rearrange("h p -> p h"))
        nc.sync.dma_start(nwq_sb[:], nwq[:].rearrange("h p -> p h"))
        nc.sync.dma_start(nwk_sb[:], nwk[:].rearrange("h p -> p h"))
        # biases as partition-broadcast tiles: added on DVE instead of via
        # K=1 ones-matmuls (removes PE instructions); loaded via ACT's queue
        bv_bc = const.tile([128, DC], F32)
        nc.scalar.dma_start(bv_bc[:], bv[:].partition_broadcast(128))
        bo_bc = const.tile([128, DIM], F32)
        nc.scalar.dma_start(bo_bc[:], bo[:].partition_broadcast(128))
        eps_sb = const.tile([128, 1], F32)
        nc.vector.memset(eps_sb[:], EPS)
        # constant logit shift: exp(x-4) keeps the biggest exp under fp8e4m3's
        # 448 max (num/denom scale identically, so softmax is unchanged)
        negc_sb = const.tile([128, 1], F32)
        nc.vector.memset(negc_sb[:], -4.0)

        v_sb = pv_pool.tile([128, SP // 128, DC], BF16)  # [s%128, s-tile, d]
        # attention-phase SBUF pools live at top scope so their ranges are
        # DISJOINT from the phase-123 pools: otherwise the first etp/oT
        # allocations inherit the freed rawp/stat space and silently wait for
        # the rope tail to finish reading it (a ~25us hidden barrier)
        aw = top.enter_context(tc.tile_pool(name=P("aw"), bufs=2))
        expp = top.enter_context(tc.tile_pool(name=P("expp"), bufs=4))
        late = top.enter_context(tc.tile_pool(name=P("late"), bufs=1))
        qT = [late.tile([128, SP], F32R, name=f"qT{h}_{rep}") for h in range(HPC)]
        kTt = [late.tile([128, SP], F32R, name=f"kTt{h}_{rep}") for h in range(HPC)]


        # single merged AllReduce: layout [1, half*2048 + qk*1024 + window]
        ar_in = dram.tile([1, 2 * SP], F32, name=f"ar_in_{rep}")
        ar_out = dram.tile([1, 2 * SP], F32, addr_space="Shared",
                           name=f"ar_out_{rep}")

        with ExitStack() as ph123:
            rawp = ph123.enter_context(tc.tile_pool(name=P("rawp"), bufs=1))
            qraw = [rawp.tile([128, SP], BF16, name=f"qraw{h}_{rep}") for h in range(HPC)]
            kraw = [rawp.tile([128, SP], BF16, name=f"kraw{h}_{rep}") for h in range(HPC)]

            stat = ph123.enter_context(tc.tile_pool(name=P("stat"), bufs=1))
            rstd_bc = [stat.tile([128, SP], F32, name=f"rstdbc{i}_{rep}")
                       for i in range(2)]
            cos_sb = stat.tile([DH, SP], F32)
            sin_sb = stat.tile([DH, SP], F32)
            strip = ph123.enter_context(tc.tile_pool(name=P("strip"), bufs=1))
            rwork = ph123.enter_context(tc.tile_pool(name=P("rwork"), bufs=1))


            def rstd_prep(m):
                """sqrt+recip+broadcast for seq half m of the merged AllReduce:
                one broadcast DMA + one Sqrt for the whole half, k recip first
                (the k ropes gate the first attention group)."""
                sj = slice(m * 1024, (m + 1) * 1024)
                for i in (1, 0):                        # k first, then q
                    bsum = strip.tile([128, 1024], F32, name="bsum")
                    o = m * 2048 + i * 1024
                    nc.sync.dma_start(
                        bsum[:], ar_out[:, o:o + 1024].partition_broadcast(128))
                    nc.scalar.activation(bsum[:], bsum[:],
                                         mybir.ActivationFunctionType.Sqrt,
                                         bias=eps_sb[:], scale=1.0 / DIM)
                    nc.vector.reciprocal(rstd_bc[i][:, sj], bsum[:])

            def rope_one(raw, dst, rbc, nw, h, m, eng=None, tag=""):
                """norm + RoPE for seq half m (1024 wide); norm-weight and
                rstd muls fused into one scalar_tensor_tensor. k ropes run on
                DVE, q ropes on GpSimd so the k chain (which gates each
                attention group) is as short as possible."""
                eng = eng or nc.vector
                sj = slice(m * 1024, (m + 1) * 1024)
                xr = rwork.tile([128, 1024], F32, name=f"xr{tag}")
                xs = rwork.tile([128, 1024], F32, name=f"xs{tag}")
                eng.scalar_tensor_tensor(
                    xr[:], raw[h][:, sj], nw[:, h:h + 1], rbc[:, sj],
                    mybir.AluOpType.mult, mybir.AluOpType.mult)
                eng.tensor_copy(xs[0:64, :], xr[64:128, :])
                eng.tensor_copy(xs[64:128, :], xr[0:64, :])
                eng.tensor_mul(xr[:], xr[:], cos_sb[:, sj])
                eng.tensor_mul(xs[:], xs[:], sin_sb[:, sj])
                eng.tensor_add(dst[h][:, sj], xr[:], xs[:])
                if m == 1:
                    eng.memset(dst[h][:, S:SP].bitcast(F32), 0.0)

            # ---------- phase 1: QKV + sumsq per 256-chunk, ARs per half ----
            with ExitStack() as ph1:
                wpool = ph1.enter_context(tc.tile_pool(name=P("wpool"), bufs=1))
                hid = ph1.enter_context(tc.tile_pool(name=P("hid"), bufs=1))
                psA = ph1.enter_context(tc.tile_pool(name=P("psA"), bufs=2, space="PSUM"))

                wq_sb = wpool.tile([128, KT, DC], BF16)
                wk_sb = wpool.tile([128, KT, DC], BF16)
                wv_sb = wpool.tile([128, KT, DC], BF16)
                nc.sync.dma_start(wq_sb[:], wqT[:])
                nc.sync.dma_start(wk_sb[:], wkT[:])



                # q/k + sum-of-squares for all chunks first, so the merged
                # AllReduce fires as early as possible; v fills PE during the
                # AllReduce flight
                for j in range(4):
                    sj = slice(j * 512, (j + 1) * 512)
                    hch = hid.tile([128, KT, 512], BF16, name="hch", bufs=2)
                    nc.sync.dma_start(hch[:], hidT[j])
                    sq_q, sq_k = {}, {}
                    for (wsb, raw, sqs, bias) in (
                            (wq_sb, qraw, sq_q, bq_sb),
                            (wk_sb, kraw, sq_k, bk_sb)):
                        for h in range(HPC):
                            pq = psA.tile([128, 512], F32, name="pq")
                            for t in range(KT):
                                nc.tensor.matmul(
                                    pq[:],
                                    wsb[:, t, h * DH:(h + 1) * DH],
                                    hch[:, t, :],
                                    start=(t == 0), stop=(t == KT - 1))
                            nc.scalar.activation(
                                raw[h][:, sj], pq[:],
                                mybir.ActivationFunctionType.Identity,
                                bias=bias[:, h:h + 1], scale=1.0 / WSCALE)
                            sqt = rwork.tile([128, 512], BF16,
                                             name=f"sq{h}", bufs=1)
                            sqs[h] = sqt
                            nc.scalar.activation(
                                sqt[:], pq[:],
                                mybir.ActivationFunctionType.Square,
                                bias=bias[:, h:h + 1], scale=1.0 / WSCALE)

                    # partial sum-of-squares -> merged AllReduce input strip
                    # (squares come from a second ACT drain of the QKV psum,
                    # so no DVE round-trip sits on the AllReduce path)
                    for idx, sqs in ((0, sq_q), (1, sq_k)):
                        pss = psA.tile([1, 512], F32, name="pss")
                        for h in range(HPC):
                            nc.tensor.matmul(pss[:], ones_col[:],
                                             sqs[h][:],
                                             start=(h == 0),
                                             stop=(h == HPC - 1))
                        # ACT drains the strip: it is idle here and sits
                        # right after this chunk's q/k drains in-order
                        ssv = rwork.tile([1, 512], F32, name="ssv")
                        nc.scalar.activation(ssv[:], pss[:],
                                             mybir.ActivationFunctionType.Identity)
                        o = (j // 2) * 2048 + idx * 1024 + (j % 2) * 512
                        nc.sync.dma_start(ar_in[:, o:o + 512], ssv[:])

                if stage >= 2:
                    if ag_mode != 'nocoll':
                        nc.gpsimd.collective_compute(
                            "AllReduce", mybir.AluOpType.add,
                            replica_groups=rg,
                            ins=[ar_in[:].opt()], outs=[ar_out[:].opt()])
                    else:
                        nc.sync.dma_start(ar_out[:], ar_in[:])

                nc.sync.dma_start(wv_sb[:], wvT[:])

                # v: natural layout, hiddenT as stationary; ACT un-scales
                # the fp8 weight prescale, GpSimd adds the bias. Runs on PE
                # during the AllReduce flight.
                for j in range(4):
                    vh = hid.tile([128, KT, 512], BF16, name="hch", bufs=2)
                    for vp in range(2):
                        nc.sync.dma_start(vh[:, 8 * vp:8 * vp + 8, :],
                                          hidT[j, :, 8 * vp:8 * vp + 8, :])
                    for st in range(4):
                        gst = j * 4 + st
                        pvp = psA.tile([128, DC], F32, name="pvp")
                        for t in range(KT):
                            nc.tensor.matmul(
                                pvp[:],
                                vh[:, t, st * 128:(st + 1) * 128],
                                wv_sb[:, t, :],
                                start=(t == 0), stop=(t == KT - 1))
                        vtmp = rwork.tile([128, DC], F32, name="vtmp")
                        nc.scalar.activation(
                            vtmp[:], pvp[:],
                            mybir.ActivationFunctionType.Identity,
                            scale=1.0 / WSCALE)
                        if gst == SP // 128 - 1:
                            nc.gpsimd.memset(v_sb[:, gst, :], 0.0)
                            nv = S - (SP // 128 - 1) * 128
                            nc.gpsimd.tensor_add(v_sb[0:nv, gst, :],
                                                 vtmp[0:nv, :], bv_bc[0:nv, :])
                        else:
                            nc.gpsimd.tensor_add(v_sb[:, gst, :], vtmp[:],
                                                 bv_bc[:])

                if stage >= 2:
                    # ropes overlap the AllReduce flight and v-tail; ordered
                    # so each attention group's k (full seq) and q (its half)
                    # unblock in sequence
                    nc.sync.dma_start(cos_sb[:], cosT[:])
                    nc.sync.dma_start(sin_sb[:], sinT[:])
                    rstd_prep(0)
                    rstd_prep(1)
                    # scores over k-half-0 tiles only need (k h m0, q h m0):
                    # subtile deps let each attention group start after two
                    # rope calls while the rest stream in behind
                    rope_one(kraw, kTt, rstd_bc[1], nwk_sb, 0, 0)
                    rope_one(qraw, qT, rstd_bc[0], nwq_sb, 0, 0)
                    rope_one(kraw, kTt, rstd_bc[1], nwk_sb, 0, 1)
                    rope_one(kraw, kTt, rstd_bc[1], nwk_sb, 1, 0)
                    rope_one(qraw, qT, rstd_bc[0], nwq_sb, 1, 0)
                    rope_one(kraw, kTt, rstd_bc[1], nwk_sb, 1, 1)
                    rope_one(qraw, qT, rstd_bc[0], nwq_sb, 0, 1)
                    rope_one(qraw, qT, rstd_bc[0], nwq_sb, 1, 1)
                if DEBUG_DUMP == "qraw":
                    nc.sync.dma_start(dbg[:], qraw[0][:])
                elif DEBUG_DUMP == "kTt":
                    nc.sync.dma_start(dbg[:], kTt[0][:])
                elif DEBUG_DUMP == "qT":
                    nc.sync.dma_start(dbg[:], qT[0][:])
                elif DEBUG_DUMP == "vsb":
                    nc.sync.dma_start(
                        dbg[:], v_sb[:].rearrange("p t d -> p (t d)"))


        if stage < 3:
            return
        # ---- phase 4+5: attention, per-(half,head) AllToAll, token proj ----
        with ExitStack() as ph4:
            oT = [aw.tile([128, SP], BF16, name=f"oT{h}_{rep}", bufs=1)
                  for h in range(HPC)]

            # AllToAll buffers, one per (half, head): input seg m = my o for
            # that head, tokens [half*1024 + m*128, +128); output seg c =
            # core c's head-h rows for MY tokens. kt tile (2c + h) of the
            # regathered och comes from atohh[half][h][c].
            agi = [[dram.tile([N_CORES, DH, 128], BF16, name=f"agi{m}{h}_{rep}")
                    for h in range(HPC)] for m in range(2)]
            ato = [[dram.tile([N_CORES, DH, 128], BF16, name=f"ato{m}{h}_{rep}")
                    for h in range(HPC)] for m in range(2)]

            with ExitStack() as attn_ps:
                psC = attn_ps.enter_context(
                    tc.tile_pool(name=P("psC"), bufs=2, space="PSUM"))
                psD = attn_ps.enter_context(
                    tc.tile_pool(name=P("psD"), bufs=1, space="PSUM"))

                for half in range(2):
                    sjs = [slice((half * 2 + jj) * 512,
                                 (half * 2 + jj + 1) * 512) for jj in range(2)]
                    for h in range(HPC):
                        # kt-outer over both chunks of the half: k/v
                        # stationary tiles serve 2 moving passes each. PV and
                        # the softmax denominator run as fp8 DoubleRow (2 key
                        # tiles per inst); scores stay f32r (contraction is a
                        # single 128 tile). po double-buffered so the next
                        # head's PV never waits this head's divide tail.
                        pos = [psD.tile([128, 512], F32, name=f"po{jj}")
                               for jj in range(2)]
                        dens = [psD.tile([128, 512], F32, name=f"den{jj}")
                                for jj in range(2)]
                        for kt in range(KT):
                            ets = []
                            for jj in range(2):
                                et = expp.tile([128, 512], BF16, name="et")
                                pscore = psC.tile([128, 512], F32,
                                                  name="pscore")
                                nc.tensor.matmul(
                                    pscore[:],
                                    kTt[h][:, kt * 128:(kt + 1) * 128],
                                    qT[h][:, sjs[jj]],
                                    start=True, stop=True)
                                nc.scalar.activation(
                                    et[:], pscore[:],
                                    mybir.ActivationFunctionType.Exp,
                                    scale=inv_sqrt_dh, bias=negc_sb[:])
                                ets.append(et)
                            for jj in range(2):
                                nc.tensor.matmul(
                                    pos[jj][:],
                                    v_sb[:, kt, h * DH:(h + 1) * DH],
                                    ets[jj][:],
                                    start=(kt == 0), stop=(kt == KT - 1))
                                nc.tensor.matmul(
                                    dens[jj][:], ones_den[:], ets[jj][:],
                                    start=(kt == 0), stop=(kt == KT - 1))
                        for jj in range(2):
                            # dens is already broadcast across partitions:
                            # the divide tail is just recip + mul on DVE
                            # (free once the ropes finish)
                            rec = aw.tile([128, 512], F32, name="rec", bufs=1)
                            nc.vector.reciprocal(rec[:], dens[jj][:])
                            nc.vector.tensor_mul(oT[h][:, sjs[jj]],
                                                 pos[jj][:], rec[:])
                        # stage this head's token segments and re-shard at
                        # once: the AllToAll overlaps the remaining attention
                        for jj in range(2):
                            j = half * 2 + jj
                            nc.sync.dma_start(
                                agi[half][h][jj * 4:(jj + 1) * 4, :, :]
                                .rearrange("s p t -> p s t"),
                                oT[h][:, j * 512:(j + 1) * 512]
                                .rearrange("p (s t) -> p s t", t=128))
                        if ag_mode != 'nocoll':
                            nc.gpsimd.collective_compute(
                                "AllToAll", mybir.AluOpType.bypass,
                                replica_groups=rg,
                                ins=[agi[half][h][:].opt()],
                                outs=[ato[half][h][:].opt()])
                        else:
                            nc.gpsimd.dma_start(ato[half][h][:],
                                                agi[half][h][:])

            if DEBUG_DUMP == "oT":
                nc.sync.dma_start(dbg[:], oT[0][:])
            if stage >= 4:
                # token-sharded projection, after attention's PSUM pools
                # close: 4 psum banks accumulate all 4 column groups with the
                # och stationary reused; contraction split by head parity so
                # the even-kt half starts before the head-1 AllToAll lands
                psE = ph4.enter_context(
                    tc.tile_pool(name=P("psE"), bufs=1, space="PSUM"))
                for half in range(2):
                    och = aw.tile([128, KT, 128], BF16, name="och", bufs=1)
                    for h in range(HPC):
                        nc.sync.dma_start(
                            och[:, h::2, :],
                            ato[half][h][:].rearrange("c p t -> p c t"))
                    for g in range(8):
                        # wo streamed per 256-column group (it would not fit
                        # resident at bf16); loads hide under attention
                        wog = aw.tile([128, KT, 256], BF16, name="wog")
                        nc.sync.dma_start(wog[:],
                                          woT[:, :, g * 256:(g + 1) * 256])
                        pout = psE.tile([128, 256], F32, name="pout", bufs=2)
                        for par in range(2):
                            for tt in range(KT // 2):
                                t = 2 * tt + par
                                nc.tensor.matmul(
                                    pout[:], och[:, t, :], wog[:, t, :],
                                    start=(par == 0 and tt == 0),
                                    stop=(par == 1 and tt == KT // 2 - 1))
                        ot = aw.tile([128, 256], F32, name="ot")
                        nc.vector.tensor_add(ot[:], pout[:],
                                             bo_bc[:, g * 256:(g + 1) * 256])
                        nc.sync.dma_start(
                            outTok[half, :, g * 256:(g + 1) * 256], ot[:])

    with tile.TileContext(nc) as tc:
        for rep in range(repeat):
            with ExitStack() as top:
                emit(tc, top, rep)

    nc.compile()
    return nc


def _prep_inputs(hidden_states, freqs_cos, freqs_sin, wq, bq, wk, bk, wv, bv,
                 norm_q_w, norm_k_w, wo, bo):
    """Host-side shard + layout prep. Returns in_maps for 8 cores."""
    f32 = np.float32
    hid = np.ascontiguousarray(np.asarray(hidden_states)[0].T, dtype=f32)
    hidT = np.zeros((DIM, SP), dtype=f32)
    hidT[:, :S] = hid
    import ml_dtypes
    bf16 = ml_dtypes.bfloat16
    fp8 = ml_dtypes.float8_e4m3fn
    # pre-tile to [chunk j, partition p, ktile t, col c]: d = t*128+p, s = j*512+c
    hidT = np.ascontiguousarray(
        hidT.reshape(KT, 128, SP // 512, 512).transpose(2, 1, 0, 3)).astype(bf16)

    def tile_w(wT, dt=bf16, scale=1.0):   # [DIM, DC] -> [128, KT, DC]
        return np.ascontiguousarray(
            (wT * scale).reshape(KT, 128, DC).transpose(1, 0, 2)).astype(dt)

    # RoPE tables: c_j[s] = cos[0,s,0,2j], s_j[s] = sin[0,s,0,2j+1]; stack [t;t]
    c = np.asarray(freqs_cos)[0, :, 0, 0::2].astype(f32).T          # [64, S]
    s = np.asarray(freqs_sin)[0, :, 0, 1::2].astype(f32).T          # [64, S]
    cosT = np.zeros((DH, SP), dtype=f32)
    sinT = np.zeros((DH, SP), dtype=f32)
    cosT[0:64, :S] = c
    cosT[64:128, :S] = c
    sinT[0:64, :S] = -s
    sinT[64:128, :S] = s

    perm = np.concatenate([np.arange(0, DH, 2), np.arange(1, DH, 2)])
    wq = np.asarray(wq)
    wk = np.asarray(wk)
    wv = np.asarray(wv)
    wo = np.asarray(wo)
    bqv = np.asarray(bq)
    bkv = np.asarray(bk)
    bvv = np.asarray(bv)
    bov = np.asarray(bo)
    nq = np.asarray(norm_q_w)
    nk = np.asarray(norm_k_w)

    # full wo, transposed+tiled: woT[p, t, j] = wo[j, t*128+p]; replicated
    woT_full = np.ascontiguousarray(
        wo.astype(f32).T.reshape(KT, 128, DIM).transpose(1, 0, 2)).astype(bf16)
    bo_full = bov.astype(f32).reshape(1, DIM)

    in_maps = []
    for core in range(N_CORES):
        rows = slice(core * DC, (core + 1) * DC)

        def permuted(mat_rows):                                     # [DC, DIM]
            blocks = [mat_rows[h * DH:(h + 1) * DH][perm] for h in range(HPC)]
            return np.concatenate(blocks, axis=0)

        def permuted_vec(vec_rows):                                 # [HPC, DH]
            blocks = [vec_rows[h * DH:(h + 1) * DH][perm] for h in range(HPC)]
            return np.stack(blocks, axis=0)

        wq_c = permuted(wq[rows].astype(f32))
        wk_c = permuted(wk[rows].astype(f32))
        in_maps.append({
            "hidT": hidT,
            "wqT": tile_w(np.ascontiguousarray(wq_c.T)),
            "wkT": tile_w(np.ascontiguousarray(wk_c.T)),
            "wvT": tile_w(np.ascontiguousarray(wv[rows].astype(f32).T)),
            "woT": woT_full,
            "cosT": cosT,
            "sinT": sinT,
            "bq": permuted_vec(bqv[rows].astype(f32)),
            "bk": permuted_vec(bkv[rows].astype(f32)),
            "bv": bvv[rows].astype(f32).reshape(1, DC),
            "bo": bo_full,
            "nwq": permuted_vec(nq[rows].astype(f32)),
            "nwk": permuted_vec(nk[rows].astype(f32)),
            "ones8": np.ones((128, 128), dtype=bf16),
        })
    return in_maps


def _assemble(results):
    """results[core]["outTok"] is [2, 128, DIM]: token blocks
    [m*128,(m+1)*128) and [1024+m*128, 1024+(m+1)*128)."""
    out = np.empty((1, S, DIM), dtype=np.float32)
    for m in range(N_CORES):
        blk = results[m]["outTok"]
        out[0, m * 128:(m + 1) * 128, :] = blk[0]
        lo = 1024 + m * 128
        n = min(128, S - lo)
        if n > 0:
            out[0, lo:lo + n, :] = blk[1][:n]
    return out


def kernel(**inputs):
    global _COMPILED
    if _COMPILED is None:
        _COMPILED = _build()
    nc = _COMPILED
    in_maps = _prep_inputs(**inputs)
    res = run_bass_kernel_spmd(nc, in_maps, core_ids=list(range(N_CORES)))
    return _assemble(res.results)



# revision 93
# speedup vs baseline: 1.0031x; 1.0031x over previous
"""CPWanSelfAttention on 8 Trainium2 NeuronCores.

Strategy: tensor-parallel over heads (16 heads -> 2 per core).
Per core c (heads 2c, 2c+1):
  - qT/kT = wq_c @ hiddenT in transposed-per-head layout [dhead, S] (bf16
    inputs), head dims host-permuted to [evens..., odds...] so RoPE pair
    math becomes aligned half-tile ops.
  - v in natural [S, d] layout (PV stationary operand).
  - RMS norm: per-core partial sum-of-squares via a second ACT Square drain
    of the QKV psum -> ONE merged AllReduce [1, 2*S] -> rstd via
    partition-broadcast DMA + Sqrt + reciprocal; RoPE fused with the norm
    scaling (scalar_tensor_tensor) on DVE, f32 intermediates, f32r q/k.
  - scoresT[k, q] = k @ qT per head in f32r; exp on ScalarE with a constant
    -4 logit shift (numerator and denominator scale identically); the
    denominator accumulates on PE via an all-ones [128,128] stationary so it
    lands pre-broadcast; divide tail (recip+mul) on DVE.
  - Per-head outputs oT (bf16) are re-sharded by TOKEN via four AllToAlls
    (one per seq half x head, issued as each head finishes so they pipeline
    on the collective engine): core m ends up with all 16 heads' o for its
    token blocks {m*128..} and {1024+m*128..}. Each core then computes the
    FULL 2048-dim output projection for its 256 tokens against streamed
    bf16 wo column-groups, contraction split by head parity so the even half
    starts before the last AllToAll lands. Host re-interleaves token blocks.
    (vs AllGather of o: 8x less collective traffic, no 16MB regather DMA.)
Engine placement is deliberate: ropes on DVE, v-bias adds + collectives on
GpSimd, strip/och/outTok DMAs on SP, sumsq drains on ACT - keeping each
in-order queue free of cross-phase stalls. Attention-phase SBUF pools are
allocated at top scope so they never alias phase-1 buffers (aliasing makes
the first attention tiles wait for the rope tail).
"""

from contextlib import ExitStack

import numpy as np
import concourse.bass as bass
import concourse.mybir as mybir
import concourse.tile as tile
from concourse import bacc
from concourse.bass_utils import run_bass_kernel_spmd

N_CORES = 8
S = 1992
SP = 2048          # seq padded to multiple of 128 (nki flash attention contract)
DIM = 2048
NHEADS = 16
DH = 128
HPC = NHEADS // N_CORES   # heads per core = 2
DC = DH * HPC             # out dims per core = 256
KT = DIM // 128           # 16 contraction tiles
NCH = SP // 512           # 4 seq chunks of 512
EPS = 1e-6

F32 = mybir.dt.float32
F32R = mybir.dt.float32r
BF16 = mybir.dt.bfloat16
F8 = mybir.dt.float8e4
DR = mybir.MatmulPerfMode.DoubleRow
WSCALE = 1.0       # no prescale needed at bf16

_COMPILED = None
DEBUG_DUMP = None


def _build(ag_mode='ato2', repeat=1, stage=4):
    nc = bacc.Bacc("TRN2", target_bir_lowering=False, debug=False,
                   num_devices=N_CORES)

    # ---- DRAM I/O (per-core shards; float32r tensors feed matmuls) ----
    hidT = nc.dram_tensor("hidT", [SP // 512, 128, KT, 512], BF16, kind="ExternalInput")
    wqT = nc.dram_tensor("wqT", [128, KT, DC], BF16, kind="ExternalInput")
    wkT = nc.dram_tensor("wkT", [128, KT, DC], BF16, kind="ExternalInput")
    wvT = nc.dram_tensor("wvT", [128, KT, DC], BF16, kind="ExternalInput")
    woT = nc.dram_tensor("woT", [128, KT, DIM], BF16, kind="ExternalInput")
    cosT = nc.dram_tensor("cosT", [DH, SP], F32, kind="ExternalInput")  # [c;c]
    sinT = nc.dram_tensor("sinT", [DH, SP], F32, kind="ExternalInput")  # [-s;s]
    bq = nc.dram_tensor("bq", [HPC, DH], F32, kind="ExternalInput")
    bk = nc.dram_tensor("bk", [HPC, DH], F32, kind="ExternalInput")
    bv = nc.dram_tensor("bv", [1, DC], F32, kind="ExternalInput")
    bo = nc.dram_tensor("bo", [1, DIM], F32, kind="ExternalInput")
    nwq = nc.dram_tensor("nwq", [HPC, DH], F32, kind="ExternalInput")
    nwk = nc.dram_tensor("nwk", [HPC, DH], F32, kind="ExternalInput")
    ones8 = nc.dram_tensor("ones8", [128, 128], BF16, kind="ExternalInput")
    # per-core output: 2 token blocks of 128 (one per seq half) x full dim
    outTok = nc.dram_tensor("outTok", [2, 128, DIM], F32, kind="ExternalOutput")
    dbg = None
    if DEBUG_DUMP in ("qraw", "kTt", "oT", "qT"):
        dbg = nc.dram_tensor("dbg", [128, SP], BF16, kind="ExternalOutput")
    elif DEBUG_DUMP == "vsb":
        dbg = nc.dram_tensor("dbg", [128, SP // 128 * DC], F8,
                             kind="ExternalOutput")

    rg = [list(range(N_CORES))]
    inv_sqrt_dh = 1.0 / float(np.sqrt(DH))

    def emit(tc, top, rep):
        P = lambda nm: f"{nm}_{rep}"
        const = top.enter_context(tc.tile_pool(name=P("const"), bufs=1))
        pv_pool = top.enter_context(tc.tile_pool(name=P("pv_pool"), bufs=1))
        dram = top.enter_context(tc.tile_pool(name=P("dram"), bufs=1, space="DRAM"))

        ones_col = const.tile([128, 1], BF16)
        nc.vector.memset(ones_col[:], 1.0)
        # [128,128] of ones: the denominator matmul then yields the softmax
        # denominator already broadcast across all partitions
        ones_den = const.tile([128, 128], BF16)
        nc.sync.dma_start(ones_den[:], ones8[:])
        ones_1row = const.tile([1, 128], F32R)
        nc.vector.memset(ones_1row[:].bitcast(F32), 1.0)
        bq_sb = const.tile([128, HPC], F32)
        bk_sb = const.tile([128, HPC], F32)
        nwq_sb = const.tile([128, HPC], F32)
        nwk_sb = const.tile([128, HPC], F32)
        nc.sync.dma_start(bq_sb[:], bq[:].rearrange("h p -> p h"))
        nc.sync.dma_start(bk_sb[:], bk[:].rearrange("h p -> p h"))
        nc.sync.dma_start(nwq_sb[:], nwq[:].rearrange("h p -> p h"))
        nc.sync.dma_start(nwk_sb[:], nwk[:].rearrange("h p -> p h"))
        # biases as partition-broadcast tiles: added on DVE instead of via
        # K=1 ones-matmuls (removes PE instructions); loaded via ACT's queue
        bv_bc = const.tile([128, DC], F32)
        nc.scalar.dma_start(bv_bc[:], bv[:].partition_broadcast(128))
        bo_bc = const.tile([128, DIM], F32)
        nc.scalar.dma_start(bo_bc[:], bo[:].partition_broadcast(128))
        eps_sb = const.tile([128, 1], F32)
        nc.vector.memset(eps_sb[:], EPS)
        # constant logit shift: exp(x-4) keeps the biggest exp under fp8e4m3's
        # 448 max (num/denom scale identically, so softmax is unchanged)
        negc_sb = const.tile([128, 1], F32)
        nc.vector.memset(negc_sb[:], -4.0)

        v_sb = pv_pool.tile([128, SP // 128, DC], BF16)  # [s%128, s-tile, d]
        # attention-phase SBUF pools live at top scope so their ranges are
        # DISJOINT from the phase-123 pools: otherwise the first etp/oT
        # allocations inherit the freed rawp/stat space and silently wait for
        # the rope tail to finish reading it (a ~25us hidden barrier)
        aw = top.enter_context(tc.tile_pool(name=P("aw"), bufs=2))
        expp = top.enter_context(tc.tile_pool(name=P("expp"), bufs=4))
        late = top.enter_context(tc.tile_pool(name=P("late"), bufs=1))
        qT = [late.tile([128, SP], F32R, name=f"qT{h}_{rep}") for h in range(HPC)]
        kTt = [late.tile([128, SP], F32R, name=f"kTt{h}_{rep}") for h in range(HPC)]


        # single merged AllReduce: layout [1, half*2048 + qk*1024 + window]
        ar_in = dram.tile([1, 2 * SP], F32, name=f"ar_in_{rep}")
        ar_out = dram.tile([1, 2 * SP], F32, addr_space="Shared",
                           name=f"ar_out_{rep}")

        with ExitStack() as ph123:
            rawp = ph123.enter_context(tc.tile_pool(name=P("rawp"), bufs=1))
            qraw = [rawp.tile([128, SP], BF16, name=f"qraw{h}_{rep}") for h in range(HPC)]
            kraw = [rawp.tile([128, SP], BF16, name=f"kraw{h}_{rep}") for h in range(HPC)]

            stat = ph123.enter_context(tc.tile_pool(name=P("stat"), bufs=1))
            rstd_bc = [stat.tile([128, SP], F32, name=f"rstdbc{i}_{rep}")
                       for i in range(2)]
            cos_sb = stat.tile([DH, SP], F32)
            sin_sb = stat.tile([DH, SP], F32)
            strip = ph123.enter_context(tc.tile_pool(name=P("strip"), bufs=1))
            rwork = ph123.enter_context(tc.tile_pool(name=P("rwork"), bufs=1))


            def rstd_prep(m):
                """sqrt+recip+broadcast for seq half m of the merged AllReduce:
                one broadcast DMA + one Sqrt for the whole half, k recip first
                (the k ropes gate the first attention group)."""
                sj = slice(m * 1024, (m + 1) * 1024)
                for i in (1, 0):                        # k first, then q
                    bsum = strip.tile([128, 1024], F32, name="bsum")
                    o = m * 2048 + i * 1024
                    nc.sync.dma_start(
                        bsum[:], ar_out[:, o:o + 1024].partition_broadcast(128))
                    nc.scalar.activation(bsum[:], bsum[:],
                                         mybir.ActivationFunctionType.Sqrt,
                                         bias=eps_sb[:], scale=1.0 / DIM)
                    nc.vector.reciprocal(rstd_bc[i][:, sj], bsum[:])

            def rope_one(raw, dst, rbc, nw, h, m, eng=None, tag=""):
                """norm + RoPE for seq half m (1024 wide); norm-weight and
                rstd muls fused into one scalar_tensor_tensor. k ropes run on
                DVE, q ropes on GpSimd so the k chain (which gates each
                attention group) is as short as possible."""
                eng = eng or nc.vector
                sj = slice(m * 1024, (m + 1) * 1024)
                xr = rwork.tile([128, 1024], F32, name=f"xr{tag}")
                xs = rwork.tile([128, 1024], F32, name=f"xs{tag}")
                eng.scalar_tensor_tensor(
                    xr[:], raw[h][:, sj], nw[:, h:h + 1], rbc[:, sj],
                    mybir.AluOpType.mult, mybir.AluOpType.mult)
                eng.tensor_copy(xs[0:64, :], xr[64:128, :])
                eng.tensor_copy(xs[64:128, :], xr[0:64, :])
                eng.tensor_mul(xr[:], xr[:], cos_sb[:, sj])
                eng.tensor_mul(xs[:], xs[:], sin_sb[:, sj])
                eng.tensor_add(dst[h][:, sj], xr[:], xs[:])
                if m == 1:
                    eng.memset(dst[h][:, S:SP].bitcast(F32), 0.0)

            # ---------- phase 1: QKV + sumsq per 256-chunk, ARs per half ----
            with ExitStack() as ph1:
                wpool = ph1.enter_context(tc.tile_pool(name=P("wpool"), bufs=1))
                hid = ph1.enter_context(tc.tile_pool(name=P("hid"), bufs=1))
                psA = ph1.enter_context(tc.tile_pool(name=P("psA"), bufs=2, space="PSUM"))

                wq_sb = wpool.tile([128, KT, DC], BF16)
                wk_sb = wpool.tile([128, KT, DC], BF16)
                wv_sb = wpool.tile([128, KT, DC], BF16)
                nc.sync.dma_start(wq_sb[:], wqT[:])
                nc.sync.dma_start(wk_sb[:], wkT[:])



                # q/k + sum-of-squares for all chunks first, so the merged
                # AllReduce fires as early as possible; v fills PE during the
                # AllReduce flight
                for j in range(4):
                    sj = slice(j * 512, (j + 1) * 512)
                    hch = hid.tile([128, KT, 512], BF16, name="hch", bufs=2)
                    nc.sync.dma_start(hch[:], hidT[j])
                    sq_q, sq_k = {}, {}
                    for (wsb, raw, sqs, bias) in (
                            (wq_sb, qraw, sq_q, bq_sb),
                            (wk_sb, kraw, sq_k, bk_sb)):
                        for h in range(HPC):
                            pq = psA.tile([128, 512], F32, name="pq")
                            for t in range(KT):
                                nc.tensor.matmul(
                                    pq[:],
                                    wsb[:, t, h * DH:(h + 1) * DH],
                                    hch[:, t, :],
                                    start=(t == 0), stop=(t == KT - 1))
                            nc.scalar.activation(
                                raw[h][:, sj], pq[:],
                                mybir.ActivationFunctionType.Identity,
                                bias=bias[:, h:h + 1], scale=1.0 / WSCALE)
                            sqt = rwork.tile([128, 512], BF16,
                                             name=f"sq{h}", bufs=1)
                            sqs[h] = sqt
                            nc.scalar.activation(
                                sqt[:], pq[:],
                                mybir.ActivationFunctionType.Square,
                                bias=bias[:, h:h + 1], scale=1.0 / WSCALE)

                    # partial sum-of-squares -> merged AllReduce input strip
                    # (squares come from a second ACT drain of the QKV psum,
                    # so no DVE round-trip sits on the AllReduce path)
                    for idx, sqs in ((0, sq_q), (1, sq_k)):
                        pss = psA.tile([1, 512], F32, name="pss")
                        for h in range(HPC):
                            nc.tensor.matmul(pss[:], ones_col[:],
                                             sqs[h][:],
                                             start=(h == 0),
                                             stop=(h == HPC - 1))
                        # ACT drains the strip: it is idle here and sits
                        # right after this chunk's q/k drains in-order
                        ssv = rwork.tile([1, 512], F32, name="ssv")
                        nc.scalar.activation(ssv[:], pss[:],
                                             mybir.ActivationFunctionType.Identity)
                        o = (j // 2) * 2048 + idx * 1024 + (j % 2) * 512
                        nc.sync.dma_start(ar_in[:, o:o + 512], ssv[:])

                if stage >= 2:
                    if ag_mode != 'nocoll':
                        nc.gpsimd.collective_compute(
                            "AllReduce", mybir.AluOpType.add,
                            replica_groups=rg,
                            ins=[ar_in[:].opt()], outs=[ar_out[:].opt()])
                    else:
                        nc.sync.dma_start(ar_out[:], ar_in[:])

                nc.sync.dma_start(wv_sb[:], wvT[:])

                # v: natural layout, hiddenT as stationary; ACT un-scales
                # the fp8 weight prescale, GpSimd adds the bias. Runs on PE
                # during the AllReduce flight.
                for j in range(4):
                    vh = hid.tile([128, KT, 512], BF16, name="hch", bufs=2)
                    for vp in range(2):
                        nc.sync.dma_start(vh[:, 8 * vp:8 * vp + 8, :],
                                          hidT[j, :, 8 * vp:8 * vp + 8, :])
                    for st in range(4):
                        gst = j * 4 + st
                        pvp = psA.tile([128, DC], F32, name="pvp")
                        for t in range(KT):
                            nc.tensor.matmul(
                                pvp[:],
                                vh[:, t, st * 128:(st + 1) * 128],
                                wv_sb[:, t, :],
                                start=(t == 0), stop=(t == KT - 1))
                        vtmp = rwork.tile([128, DC], F32, name="vtmp")
                        nc.scalar.activation(
                            vtmp[:], pvp[:],
                            mybir.ActivationFunctionType.Identity,
                            scale=1.0 / WSCALE)
                        if gst == SP // 128 - 1:
                            nc.gpsimd.memset(v_sb[:, gst, :], 0.0)
                            nv = S - (SP // 128 - 1) * 128
                            nc.gpsimd.tensor_add(v_sb[0:nv, gst, :],
                                                 vtmp[0:nv, :], bv_bc[0:nv, :])
                        else:
                            nc.gpsimd.tensor_add(v_sb[:, gst, :], vtmp[:],
                                                 bv_bc[:])

                if stage >= 2:
                    # ropes overlap the AllReduce flight and v-tail; ordered
                    # so each attention group's k (full seq) and q (its half)
                    # unblock in sequence
                    nc.sync.dma_start(cos_sb[:], cosT[:])
                    nc.sync.dma_start(sin_sb[:], sinT[:])
                    rstd_prep(0)
                    rstd_prep(1)
                    # scores over k-half-0 tiles only need (k h m0, q h m0):
                    # subtile deps let each attention group start after two
                    # rope calls while the rest stream in behind
                    rope_one(kraw, kTt, rstd_bc[1], nwk_sb, 0, 0)
                    rope_one(qraw, qT, rstd_bc[0], nwq_sb, 0, 0)
                    rope_one(kraw, kTt, rstd_bc[1], nwk_sb, 0, 1)
                    rope_one(kraw, kTt, rstd_bc[1], nwk_sb, 1, 0)
                    rope_one(qraw, qT, rstd_bc[0], nwq_sb, 1, 0)
                    rope_one(kraw, kTt, rstd_bc[1], nwk_sb, 1, 1)
                    rope_one(qraw, qT, rstd_bc[0], nwq_sb, 0, 1)
                    rope_one(qraw, qT, rstd_bc[0], nwq_sb, 1, 1)
                if DEBUG_DUMP == "qraw":
                    nc.sync.dma_start(dbg[:], qraw[0][:])
                elif DEBUG_DUMP == "kTt":
                    nc.sync.dma_start(dbg[:], kTt[0][:])
                elif DEBUG_DUMP == "qT":
                    nc.sync.dma_start(dbg[:], qT[0][:])
                elif DEBUG_DUMP == "vsb":
                    nc.sync.dma_start(
                        dbg[:], v_sb[:].rearrange("p t d -> p (t d)"))


        if stage < 3:
            return
        # ---- phase 4+5: attention, per-(half,head) AllToAll, token proj ----
        with ExitStack() as ph4:
            oT = [aw.tile([128, SP], BF16, name=f"oT{h}_{rep}", bufs=1)
                  for h in range(HPC)]

            # AllToAll buffers, one per (half, head): input seg m = my o for
            # that head, tokens [half*1024 + m*128, +128); output seg c =
            # core c's head-h rows for MY tokens. kt tile (2c + h) of the
            # regathered och comes from atohh[half][h][c].
            agi = [[dram.tile([N_CORES, DH, 128], BF16, name=f"agi{m}{h}_{rep}")
                    for h in range(HPC)] for m in range(2)]
            ato = [[dram.tile([N_CORES, DH, 128], BF16, name=f"ato{m}{h}_{rep}")
                    for h in range(HPC)] for m in range(2)]

            with ExitStack() as attn_ps:
                psC = attn_ps.enter_context(
                    tc.tile_pool(name=P("psC"), bufs=2, space="PSUM"))
                psD = attn_ps.enter_context(
                    tc.tile_pool(name=P("psD"), bufs=1, space="PSUM"))

                for half in range(2):
                    sjs = [slice((half * 2 + jj) * 512,
                                 (half * 2 + jj + 1) * 512) for jj in range(2)]
                    for h in range(HPC):
                        # kt-outer over both chunks of the half: k/v
                        # stationary tiles serve 2 moving passes each. PV and
                        # the softmax denominator run as fp8 DoubleRow (2 key
                        # tiles per inst); scores stay f32r (contraction is a
                        # single 128 tile). po double-buffered so the next
                        # head's PV never waits this head's divide tail.
                        pos = [psD.tile([128, 512], F32, name=f"po{jj}")
                               for jj in range(2)]
                        dens = [psD.tile([128, 512], F32, name=f"den{jj}")
                                for jj in range(2)]
                        for kt in range(KT):
                            ets = []
                            for jj in range(2):
                                et = expp.tile([128, 512], BF16, name="et")
                                pscore = psC.tile([128, 512], F32,
                                                  name="pscore")
                                nc.tensor.matmul(
                                    pscore[:],
                                    kTt[h][:, kt * 128:(kt + 1) * 128],
                                    qT[h][:, sjs[jj]],
                                    start=True, stop=True)
                                nc.scalar.activation(
                                    et[:], pscore[:],
                                    mybir.ActivationFunctionType.Exp,
                                    scale=inv_sqrt_dh, bias=negc_sb[:])
                                ets.append(et)
                            for jj in range(2):
                                nc.tensor.matmul(
                                    pos[jj][:],
                                    v_sb[:, kt, h * DH:(h + 1) * DH],
                                    ets[jj][:],
                                    start=(kt == 0), stop=(kt == KT - 1))
                                nc.tensor.matmul(
                                    dens[jj][:], ones_den[:], ets[jj][:],
                                    start=(kt == 0), stop=(kt == KT - 1))
                        for jj in range(2):
                            # dens is already broadcast across partitions:
                            # the divide tail is just recip + mul on DVE
                            # (free once the ropes finish)
                            rec = aw.tile([128, 512], F32, name="rec", bufs=1)
                            nc.vector.reciprocal(rec[:], dens[jj][:])
                            nc.vector.tensor_mul(oT[h][:, sjs[jj]],
                                                 pos[jj][:], rec[:])
                        # stage this head's token segments and re-shard at
                        # once: the AllToAll overlaps the remaining attention
                        for jj in range(2):
                            j = half * 2 + jj
                            nc.sync.dma_start(
                                agi[half][h][jj * 4:(jj + 1) * 4, :, :]
                                .rearrange("s p t -> p s t"),
                                oT[h][:, j * 512:(j + 1) * 512]
                                .rearrange("p (s t) -> p s t", t=128))
                        if ag_mode != 'nocoll':
                            nc.gpsimd.collective_compute(
                                "AllToAll", mybir.AluOpType.bypass,
                                replica_groups=rg,
                                ins=[agi[half][h][:].opt()],
                                outs=[ato[half][h][:].opt()])
                        else:
                            nc.gpsimd.dma_start(ato[half][h][:],
                                                agi[half][h][:])

            if DEBUG_DUMP == "oT":
                nc.sync.dma_start(dbg[:], oT[0][:])
            if stage >= 4:
                # token-sharded projection, after attention's PSUM pools
                # close: 4 psum banks accumulate all 4 column groups with the
                # och stationary reused; contraction split by head parity so
                # the even-kt half starts before the head-1 AllToAll lands
                psE = ph4.enter_context(
                    tc.tile_pool(name=P("psE"), bufs=1, space="PSUM"))
                for half in range(2):
                    och = aw.tile([128, KT, 128], BF16, name="och", bufs=1)
                    for h in range(HPC):
                        nc.sync.dma_start(
                            och[:, h::2, :],
                            ato[half][h][:].rearrange("c p t -> p c t"))
                    for g in range(8):
                        # wo streamed per 256-column group (it would not fit
                        # resident at bf16); loads hide under attention
                        wog = aw.tile([128, KT, 256], BF16, name="wog")
                        nc.sync.dma_start(wog[:],
                                          woT[:, :, g * 256:(g + 1) * 256])
                        pout = psE.tile([128, 256], F32, name="pout", bufs=2)
                        for par in range(2):
                            for tt in range(KT // 2):
                                t = 2 * tt + par
                                nc.tensor.matmul(
                                    pout[:], och[:, t, :], wog[:, t, :],
                                    start=(par == 0 and tt == 0),
                                    stop=(par == 1 and tt == KT // 2 - 1))
                        ot = aw.tile([128, 256], F32, name="ot")
                        nc.vector.tensor_add(ot[:], pout[:],
                                             bo_bc[:, g * 256:(g + 1) * 256])
                        nc.sync.dma_start(
                            outTok[half, :, g * 256:(g + 1) * 256], ot[:])

    with tile.TileContext(nc) as tc:
        for rep in range(repeat):
            with ExitStack() as top:
                emit(tc, top, rep)

    nc.compile()
    return nc


def _prep_inputs(hidden_states, freqs_cos, freqs_sin, wq, bq, wk, bk, wv, bv,
                 norm_q_w, norm_k_w, wo, bo):
    """Host-side shard + layout prep. Returns in_maps for 8 cores."""
    f32 = np.float32
    hid = np.ascontiguousarray(np.asarray(hidden_states)[0].T, dtype=f32)
    hidT = np.zeros((DIM, SP), dtype=f32)
    hidT[:, :S] = hid
    import ml_dtypes
    bf16 = ml_dtypes.bfloat16
    fp8 = ml_dtypes.float8_e4m3fn
    # pre-tile to [chunk j, partition p, ktile t, col c]: d = t*128+p, s = j*512+c
    hidT = np.ascontiguousarray(
        hidT.reshape(KT, 128, SP // 512, 512).transpose(2, 1, 0, 3)).astype(bf16)

    def tile_w(wT, dt=bf16, scale=1.0):   # [DIM, DC] -> [128, KT, DC]
        return np.ascontiguousarray(
            (wT * scale).reshape(KT, 128, DC).transpose(1, 0, 2)).astype(dt)

    # RoPE tables: c_j[s] = cos[0,s,0,2j], s_j[s] = sin[0,s,0,2j+1]; stack [t;t]
    c = np.asarray(freqs_cos)[0, :, 0, 0::2].astype(f32).T          # [64, S]
    s = np.asarray(freqs_sin)[0, :, 0, 1::2].astype(f32).T          # [64, S]
    cosT = np.zeros((DH, SP), dtype=f32)
    sinT = np.zeros((DH, SP), dtype=f32)
    cosT[0:64, :S] = c
    cosT[64:128, :S] = c
    sinT[0:64, :S] = -s
    sinT[64:128, :S] = s

    perm = np.concatenate([np.arange(0, DH, 2), np.arange(1, DH, 2)])
    wq = np.asarray(wq)
    wk = np.asarray(wk)
    wv = np.asarray(wv)
    wo = np.asarray(wo)
    bqv = np.asarray(bq)
    bkv = np.asarray(bk)
    bvv = np.asarray(bv)
    bov = np.asarray(bo)
    nq = np.asarray(norm_q_w)
    nk = np.asarray(norm_k_w)

    # full wo, transposed+tiled: woT[p, t, j] = wo[j, t*128+p]; replicated
    woT_full = np.ascontiguousarray(
        wo.astype(f32).T.reshape(KT, 128, DIM).transpose(1, 0, 2)).astype(bf16)
    bo_full = bov.astype(f32).reshape(1, DIM)

    in_maps = []
    for core in range(N_CORES):
        rows = slice(core * DC, (core + 1) * DC)

        def permuted(mat_rows):                                     # [DC, DIM]
            blocks = [mat_rows[h * DH:(h + 1) * DH][perm] for h in range(HPC)]
            return np.concatenate(blocks, axis=0)

        def permuted_vec(vec_rows):                                 # [HPC, DH]
            blocks = [vec_rows[h * DH:(h + 1) * DH][perm] for h in range(HPC)]
            return np.stack(blocks, axis=0)

        wq_c = permuted(wq[rows].astype(f32))
        wk_c = permuted(wk[rows].astype(f32))
        in_maps.append({
            "hidT": hidT,
            "wqT": tile_w(np.ascontiguousarray(wq_c.T)),
            "wkT": tile_w(np.ascontiguousarray(wk_c.T)),
            "wvT": tile_w(np.ascontiguousarray(wv[rows].astype(f32).T)),
            "woT": woT_full,
            "cosT": cosT,
            "sinT": sinT,
            "bq": permuted_vec(bqv[rows].astype(f32)),
            "bk": permuted_vec(bkv[rows].astype(f32)),
            "bv": bvv[rows].astype(f32).reshape(1, DC),
            "bo": bo_full,
            "nwq": permuted_vec(nq[rows].astype(f32)),
            "nwk": permuted_vec(nk[rows].astype(f32)),
            "ones8": np.ones((128, 128), dtype=bf16),
        })
    return in_maps


def _assemble(results):
    """results[core]["outTok"] is [2, 128, DIM]: token blocks
    [m*128,(m+1)*128) and [1024+m*128, 1024+(m+1)*128)."""
    out = np.empty((1, S, DIM), dtype=np.float32)
    for m in range(N_CORES):
        blk = results[m]["outTok"]
        out[0, m * 128:(m + 1) * 128, :] = blk[0]
        lo = 1024 + m * 128
        n = min(128, S - lo)
        if n > 0:
            out[0, lo:lo + n, :] = blk[1][:n]
    return out


def kernel(**inputs):
    global _COMPILED
    if _COMPILED is None:
        _COMPILED = _build()
    nc = _COMPILED
    in_maps = _prep_inputs(**inputs)
    res = run_bass_kernel_spmd(nc, in_maps, core_ids=list(range(N_CORES)))
    return _assemble(res.results)



# revision 94
# speedup vs baseline: 1.1114x; 1.1079x over previous
"""CPWanSelfAttention on 8 Trainium2 NeuronCores.

Strategy: tensor-parallel over heads (16 heads -> 2 per core).
Per core c (heads 2c, 2c+1):
  - qT/kT = wq_c @ hiddenT in transposed-per-head layout [dhead, S] (bf16
    inputs), head dims host-permuted to [evens..., odds...] so RoPE pair
    math becomes aligned half-tile ops.
  - v in natural [S, d] layout (PV stationary operand).
  - RMS norm: per-core partial sum-of-squares via a second ACT Square drain
    of the QKV psum -> ONE merged AllReduce [1, 2*S] -> rstd via
    partition-broadcast DMA + Sqrt + reciprocal; RoPE fused with the norm
    scaling (scalar_tensor_tensor) on DVE, f32 intermediates, f32r q/k.
  - scoresT[k, q] = k @ qT per head in f32r; exp on ScalarE with a constant
    -4 logit shift (numerator and denominator scale identically); the
    denominator accumulates on PE via an all-ones [128,128] stationary so it
    lands pre-broadcast; divide tail (recip+mul) on DVE.
  - Per-head outputs oT (bf16) are re-sharded by TOKEN via four AllToAlls
    (one per seq half x head, issued as each head finishes so they pipeline
    on the collective engine): core m ends up with all 16 heads' o for its
    token blocks {m*128..} and {1024+m*128..}. Each core then computes the
    FULL 2048-dim output projection for its 256 tokens against streamed
    bf16 wo column-groups, contraction split by head parity so the even half
    starts before the last AllToAll lands. Host re-interleaves token blocks.
    (vs AllGather of o: 8x less collective traffic, no 16MB regather DMA.)
Engine placement is deliberate: ropes on DVE, v-bias adds + collectives on
GpSimd, strip/och/outTok DMAs on SP, sumsq drains on ACT - keeping each
in-order queue free of cross-phase stalls. Attention-phase SBUF pools are
allocated at top scope so they never alias phase-1 buffers (aliasing makes
the first attention tiles wait for the rope tail).
"""

from contextlib import ExitStack

import numpy as np
import concourse.bass as bass
import concourse.mybir as mybir
import concourse.tile as tile
from concourse import bacc
from concourse.bass_utils import run_bass_kernel_spmd

N_CORES = 8
S = 1992
SP = 2048          # seq padded to multiple of 128 (nki flash attention contract)
DIM = 2048
NHEADS = 16
DH = 128
HPC = NHEADS // N_CORES   # heads per core = 2
DC = DH * HPC             # out dims per core = 256
KT = DIM // 128           # 16 contraction tiles
NCH = SP // 512           # 4 seq chunks of 512
EPS = 1e-6

F32 = mybir.dt.float32
F32R = mybir.dt.float32r
BF16 = mybir.dt.bfloat16
F8 = mybir.dt.float8e4
DR = mybir.MatmulPerfMode.DoubleRow
WSCALE = 1.0       # no prescale needed at bf16

_COMPILED = None
DEBUG_DUMP = None


def _build(ag_mode='ato2', repeat=1, stage=4):
    nc = bacc.Bacc("TRN2", target_bir_lowering=False, debug=False,
                   num_devices=N_CORES)

    # ---- DRAM I/O (per-core shards; float32r tensors feed matmuls) ----
    hidT = nc.dram_tensor("hidT", [SP // 512, 128, KT, 512], BF16, kind="ExternalInput")
    wqT = nc.dram_tensor("wqT", [128, KT, DC], BF16, kind="ExternalInput")
    wkT = nc.dram_tensor("wkT", [128, KT, DC], BF16, kind="ExternalInput")
    wvT = nc.dram_tensor("wvT", [128, KT, DC], BF16, kind="ExternalInput")
    woT = nc.dram_tensor("woT", [128, KT, DIM], BF16, kind="ExternalInput")
    cosT = nc.dram_tensor("cosT", [DH, SP], F32, kind="ExternalInput")  # [c;c]
    sinT = nc.dram_tensor("sinT", [DH, SP], F32, kind="ExternalInput")  # [-s;s]
    bq = nc.dram_tensor("bq", [HPC, DH], F32, kind="ExternalInput")
    bk = nc.dram_tensor("bk", [HPC, DH], F32, kind="ExternalInput")
    bv = nc.dram_tensor("bv", [1, DC], F32, kind="ExternalInput")
    bo = nc.dram_tensor("bo", [1, DIM], F32, kind="ExternalInput")
    nwq = nc.dram_tensor("nwq", [HPC, DH], F32, kind="ExternalInput")
    nwk = nc.dram_tensor("nwk", [HPC, DH], F32, kind="ExternalInput")
    ones8 = nc.dram_tensor("ones8", [128, 128], BF16, kind="ExternalInput")
    # per-core output: 2 token blocks of 128 (one per seq half) x full dim
    outTok = nc.dram_tensor("outTok", [2, 128, DIM], F32, kind="ExternalOutput")
    dbg = None
    if DEBUG_DUMP in ("qraw", "kTt", "oT", "qT"):
        dbg = nc.dram_tensor("dbg", [128, SP], BF16, kind="ExternalOutput")
    elif DEBUG_DUMP == "vsb":
        dbg = nc.dram_tensor("dbg", [128, SP // 128 * DC], F8,
                             kind="ExternalOutput")

    rg = [list(range(N_CORES))]
    inv_sqrt_dh = 1.0 / float(np.sqrt(DH))

    def emit(tc, top, rep):
        P = lambda nm: f"{nm}_{rep}"
        const = top.enter_context(tc.tile_pool(name=P("const"), bufs=1))
        pv_pool = top.enter_context(tc.tile_pool(name=P("pv_pool"), bufs=1))
        dram = top.enter_context(tc.tile_pool(name=P("dram"), bufs=1, space="DRAM"))

        ones_col = const.tile([128, 1], BF16)
        nc.vector.memset(ones_col[:], 1.0)
        # [128,128] of ones: the denominator matmul then yields the softmax
        # denominator already broadcast across all partitions
        ones_den = const.tile([128, 128], BF16)
        nc.sync.dma_start(ones_den[:], ones8[:])
        ones_1row = const.tile([1, 128], F32R)
        nc.vector.memset(ones_1row[:].bitcast(F32), 1.0)
        bq_sb = const.tile([128, HPC], F32)
        bk_sb = const.tile([128, HPC], F32)
        nwq_sb = const.tile([128, HPC], F32)
        nwk_sb = const.tile([128, HPC], F32)
        nc.sync.dma_start(bq_sb[:], bq[:].rearrange("h p -> p h"))
        nc.sync.dma_start(bk_sb[:], bk[:].rearrange("h p -> p h"))
        nc.sync.dma_start(nwq_sb[:], nwq[:].rearrange("h p -> p h"))
        nc.sync.dma_start(nwk_sb[:], nwk[:].rearrange("h p -> p h"))
        # biases as partition-broadcast tiles: added on DVE instead of via
        # K=1 ones-matmuls (removes PE instructions); loaded via ACT's queue
        bv_bc = const.tile([128, DC], F32)
        nc.scalar.dma_start(bv_bc[:], bv[:].partition_broadcast(128))
        bo_bc = const.tile([128, DIM], F32)
        nc.scalar.dma_start(bo_bc[:], bo[:].partition_broadcast(128))
        eps_sb = const.tile([128, 1], F32)
        nc.vector.memset(eps_sb[:], EPS)
        # constant logit shift: exp(x-4) keeps the biggest exp under fp8e4m3's
        # 448 max (num/denom scale identically, so softmax is unchanged)
        negc_sb = const.tile([128, 1], F32)
        nc.vector.memset(negc_sb[:], -4.0)

        v_sb = pv_pool.tile([128, SP // 128, DC], BF16)  # [s%128, s-tile, d]
        # attention-phase SBUF pools live at top scope so their ranges are
        # DISJOINT from the phase-123 pools: otherwise the first etp/oT
        # allocations inherit the freed rawp/stat space and silently wait for
        # the rope tail to finish reading it (a ~25us hidden barrier)
        aw = top.enter_context(tc.tile_pool(name=P("aw"), bufs=2))
        expp = top.enter_context(tc.tile_pool(name=P("expp"), bufs=4))
        late = top.enter_context(tc.tile_pool(name=P("late"), bufs=1))
        qT = [late.tile([128, SP], F32R, name=f"qT{h}_{rep}") for h in range(HPC)]
        kTt = [late.tile([128, SP], F32R, name=f"kTt{h}_{rep}") for h in range(HPC)]


        # single merged AllReduce: layout [1, half*2048 + qk*1024 + window]
        ar_in = dram.tile([1, 2 * SP], F32, name=f"ar_in_{rep}")
        ar_out = dram.tile([1, 2 * SP], F32, addr_space="Shared",
                           name=f"ar_out_{rep}")

        with ExitStack() as ph123:
            rawp = ph123.enter_context(tc.tile_pool(name=P("rawp"), bufs=1))
            qraw = [rawp.tile([128, SP], BF16, name=f"qraw{h}_{rep}") for h in range(HPC)]
            kraw = [rawp.tile([128, SP], BF16, name=f"kraw{h}_{rep}") for h in range(HPC)]

            stat = ph123.enter_context(tc.tile_pool(name=P("stat"), bufs=1))
            rstd_bc = [stat.tile([128, SP], F32, name=f"rstdbc{i}_{rep}")
                       for i in range(2)]
            cos_sb = stat.tile([DH, SP], F32)
            sin_sb = stat.tile([DH, SP], F32)
            strip = ph123.enter_context(tc.tile_pool(name=P("strip"), bufs=1))
            rwork = ph123.enter_context(tc.tile_pool(name=P("rwork"), bufs=1))


            def rstd_prep(m):
                """sqrt+recip+broadcast for seq half m of the merged AllReduce:
                one broadcast DMA + one Sqrt for the whole half, k recip first
                (the k ropes gate the first attention group)."""
                sj = slice(m * 1024, (m + 1) * 1024)
                for i in (1, 0):                        # k first, then q
                    bsum = strip.tile([128, 1024], F32, name="bsum")
                    o = m * 2048 + i * 1024
                    nc.sync.dma_start(
                        bsum[:], ar_out[:, o:o + 1024].partition_broadcast(128))
                    nc.scalar.activation(bsum[:], bsum[:],
                                         mybir.ActivationFunctionType.Sqrt,
                                         bias=eps_sb[:], scale=1.0 / DIM)
                    nc.vector.reciprocal(rstd_bc[i][:, sj], bsum[:])

            def rope_one(raw, dst, rbc, nw, h, m, eng=None, tag=""):
                """norm + RoPE for seq half m (1024 wide); norm-weight and
                rstd muls fused into one scalar_tensor_tensor. k ropes run on
                DVE, q ropes on GpSimd so the k chain (which gates each
                attention group) is as short as possible."""
                eng = eng or nc.vector
                sj = slice(m * 1024, (m + 1) * 1024)
                xr = rwork.tile([128, 1024], F32, name=f"xr{tag}")
                xs = rwork.tile([128, 1024], F32, name=f"xs{tag}")
                eng.scalar_tensor_tensor(
                    xr[:], raw[h][:, sj], nw[:, h:h + 1], rbc[:, sj],
                    mybir.AluOpType.mult, mybir.AluOpType.mult)
                eng.tensor_copy(xs[0:64, :], xr[64:128, :])
                eng.tensor_copy(xs[64:128, :], xr[0:64, :])
                eng.tensor_mul(xr[:], xr[:], cos_sb[:, sj])
                eng.tensor_mul(xs[:], xs[:], sin_sb[:, sj])
                eng.tensor_add(dst[h][:, sj], xr[:], xs[:])
                if m == 1:
                    eng.memset(dst[h][:, S:SP].bitcast(F32), 0.0)

            # ---------- phase 1: QKV + sumsq per 256-chunk, ARs per half ----
            with ExitStack() as ph1:
                wpool = ph1.enter_context(tc.tile_pool(name=P("wpool"), bufs=1))
                hid = ph1.enter_context(tc.tile_pool(name=P("hid"), bufs=1))
                psA = ph1.enter_context(tc.tile_pool(name=P("psA"), bufs=2, space="PSUM"))

                wq_sb = wpool.tile([128, KT, DC], BF16)
                wk_sb = wpool.tile([128, KT, DC], BF16)
                wv_sb = wpool.tile([128, KT, DC], BF16)
                nc.sync.dma_start(wq_sb[:], wqT[:])
                nc.sync.dma_start(wk_sb[:], wkT[:])



                # q/k + sum-of-squares for all chunks first, so the merged
                # AllReduce fires as early as possible; v fills PE during the
                # AllReduce flight
                for j in range(4):
                    sj = slice(j * 512, (j + 1) * 512)
                    hch = hid.tile([128, KT, 512], BF16, name="hch", bufs=2)
                    nc.sync.dma_start(hch[:], hidT[j])
                    sq_q, sq_k = {}, {}
                    for (wsb, raw, sqs, bias) in (
                            (wq_sb, qraw, sq_q, bq_sb),
                            (wk_sb, kraw, sq_k, bk_sb)):
                        for h in range(HPC):
                            pq = psA.tile([128, 512], F32, name="pq")
                            for t in range(KT):
                                nc.tensor.matmul(
                                    pq[:],
                                    wsb[:, t, h * DH:(h + 1) * DH],
                                    hch[:, t, :],
                                    start=(t == 0), stop=(t == KT - 1))
                            nc.scalar.activation(
                                raw[h][:, sj], pq[:],
                                mybir.ActivationFunctionType.Identity,
                                bias=bias[:, h:h + 1], scale=1.0 / WSCALE)
                            sqt = rwork.tile([128, 512], BF16,
                                             name=f"sq{h}", bufs=1)
                            sqs[h] = sqt
                            nc.scalar.activation(
                                sqt[:], pq[:],
                                mybir.ActivationFunctionType.Square,
                                bias=bias[:, h:h + 1], scale=1.0 / WSCALE)

                    # partial sum-of-squares -> merged AllReduce input strip
                    # (squares come from a second ACT drain of the QKV psum,
                    # so no DVE round-trip sits on the AllReduce path)
                    for idx, sqs in ((0, sq_q), (1, sq_k)):
                        pss = psA.tile([1, 512], F32, name="pss")
                        for h in range(HPC):
                            nc.tensor.matmul(pss[:], ones_col[:],
                                             sqs[h][:],
                                             start=(h == 0),
                                             stop=(h == HPC - 1))
                        # ACT drains the strip: it is idle here and sits
                        # right after this chunk's q/k drains in-order
                        ssv = rwork.tile([1, 512], F32, name="ssv")
                        nc.scalar.activation(ssv[:], pss[:],
                                             mybir.ActivationFunctionType.Identity)
                        o = (j // 2) * 2048 + idx * 1024 + (j % 2) * 512
                        nc.sync.dma_start(ar_in[:, o:o + 512], ssv[:])

                if stage >= 2:
                    if ag_mode != 'nocoll':
                        nc.gpsimd.collective_compute(
                            "AllReduce", mybir.AluOpType.add,
                            replica_groups=rg,
                            ins=[ar_in[:].opt()], outs=[ar_out[:].opt()])
                    else:
                        nc.sync.dma_start(ar_out[:], ar_in[:])

                nc.sync.dma_start(wv_sb[:], wvT[:])

                # v: natural layout, hiddenT as stationary; ACT un-scales
                # the fp8 weight prescale, GpSimd adds the bias. Runs on PE
                # during the AllReduce flight.
                for j in range(4):
                    vh = hid.tile([128, KT, 512], BF16, name="hch", bufs=2)
                    for vp in range(2):
                        nc.sync.dma_start(vh[:, 8 * vp:8 * vp + 8, :],
                                          hidT[j, :, 8 * vp:8 * vp + 8, :])
                    for st in range(4):
                        gst = j * 4 + st
                        pvp = psA.tile([128, DC], F32, name="pvp")
                        for t in range(KT):
                            nc.tensor.matmul(
                                pvp[:],
                                vh[:, t, st * 128:(st + 1) * 128],
                                wv_sb[:, t, :],
                                start=(t == 0), stop=(t == KT - 1))
                        vtmp = rwork.tile([128, DC], F32, name="vtmp")
                        nc.scalar.activation(
                            vtmp[:], pvp[:],
                            mybir.ActivationFunctionType.Identity,
                            scale=1.0 / WSCALE)
                        if gst == SP // 128 - 1:
                            nc.gpsimd.memset(v_sb[:, gst, :], 0.0)
                            nv = S - (SP // 128 - 1) * 128
                            nc.gpsimd.tensor_add(v_sb[0:nv, gst, :],
                                                 vtmp[0:nv, :], bv_bc[0:nv, :])
                        else:
                            nc.gpsimd.tensor_add(v_sb[:, gst, :], vtmp[:],
                                                 bv_bc[:])

                if stage >= 2:
                    # ropes overlap the AllReduce flight and v-tail; ordered
                    # so each attention group's k (full seq) and q (its half)
                    # unblock in sequence
                    nc.sync.dma_start(cos_sb[:], cosT[:])
                    nc.sync.dma_start(sin_sb[:], sinT[:])
                    rstd_prep(0)
                    rstd_prep(1)
                    # scores over k-half-0 tiles only need (k h m0, q h m0):
                    # subtile deps let each attention group start after two
                    # rope calls while the rest stream in behind
                    rope_one(kraw, kTt, rstd_bc[1], nwk_sb, 0, 0)
                    rope_one(qraw, qT, rstd_bc[0], nwq_sb, 0, 0)
                    rope_one(kraw, kTt, rstd_bc[1], nwk_sb, 0, 1)
                    rope_one(kraw, kTt, rstd_bc[1], nwk_sb, 1, 0)
                    rope_one(qraw, qT, rstd_bc[0], nwq_sb, 1, 0)
                    rope_one(kraw, kTt, rstd_bc[1], nwk_sb, 1, 1)
                    rope_one(qraw, qT, rstd_bc[0], nwq_sb, 0, 1)
                    rope_one(qraw, qT, rstd_bc[0], nwq_sb, 1, 1)
                if DEBUG_DUMP == "qraw":
                    nc.sync.dma_start(dbg[:], qraw[0][:])
                elif DEBUG_DUMP == "kTt":
                    nc.sync.dma_start(dbg[:], kTt[0][:])
                elif DEBUG_DUMP == "qT":
                    nc.sync.dma_start(dbg[:], qT[0][:])
                elif DEBUG_DUMP == "vsb":
                    nc.sync.dma_start(
                        dbg[:], v_sb[:].rearrange("p t d -> p (t d)"))


        if stage < 3:
            return
        # ---- phase 4+5: attention, per-(half,head) AllToAll, token proj ----
        with ExitStack() as ph4:
            oT = [aw.tile([128, SP], BF16, name=f"oT{h}_{rep}", bufs=1)
                  for h in range(HPC)]

            # AllToAll buffers, one per (half, head): input seg m = my o for
            # that head, tokens [half*1024 + m*128, +128); output seg c =
            # core c's head-h rows for MY tokens. kt tile (2c + h) of the
            # regathered och comes from atohh[half][h][c].
            agi = [[dram.tile([N_CORES, DH, 128], BF16, name=f"agi{m}{h}_{rep}")
                    for h in range(HPC)] for m in range(2)]
            ato = [[dram.tile([N_CORES, DH, 128], BF16, name=f"ato{m}{h}_{rep}")
                    for h in range(HPC)] for m in range(2)]

            with ExitStack() as attn_ps:
                psC = attn_ps.enter_context(
                    tc.tile_pool(name=P("psC"), bufs=4, space="PSUM"))
                psD = attn_ps.enter_context(
                    tc.tile_pool(name=P("psD"), bufs=1, space="PSUM"))

                for half in range(2):
                    sjs = [slice((half * 2 + jj) * 512,
                                 (half * 2 + jj + 1) * 512) for jj in range(2)]
                    for h in range(HPC):
                        # kt-outer over both chunks of the half: k/v
                        # stationary tiles serve 2 moving passes each. PV and
                        # the softmax denominator run as fp8 DoubleRow (2 key
                        # tiles per inst); scores stay f32r (contraction is a
                        # single 128 tile). po double-buffered so the next
                        # head's PV never waits this head's divide tail.
                        pos = [psD.tile([128, 512], F32, name=f"po{jj}")
                               for jj in range(2)]
                        dens = [psD.tile([128, 512], F32, name=f"den{jj}")
                                for jj in range(2)]
                        for kt in range(KT):
                            ets = []
                            for jj in range(2):
                                et = expp.tile([128, 512], BF16, name="et")
                                pscore = psC.tile([128, 512], F32,
                                                  name="pscore")
                                nc.tensor.matmul(
                                    pscore[:],
                                    kTt[h][:, kt * 128:(kt + 1) * 128],
                                    qT[h][:, sjs[jj]],
                                    start=True, stop=True)
                                nc.scalar.activation(
                                    et[:], pscore[:],
                                    mybir.ActivationFunctionType.Exp,
                                    scale=inv_sqrt_dh, bias=negc_sb[:])
                                ets.append(et)
                            for jj in range(2):
                                nc.tensor.matmul(
                                    pos[jj][:],
                                    v_sb[:, kt, h * DH:(h + 1) * DH],
                                    ets[jj][:],
                                    start=(kt == 0), stop=(kt == KT - 1))
                                nc.tensor.matmul(
                                    dens[jj][:], ones_den[:], ets[jj][:],
                                    start=(kt == 0), stop=(kt == KT - 1))
                        for jj in range(2):
                            # dens is already broadcast across partitions:
                            # the divide tail is just recip + mul on DVE
                            # (free once the ropes finish)
                            rec = aw.tile([128, 512], F32, name="rec", bufs=1)
                            nc.vector.reciprocal(rec[:], dens[jj][:])
                            nc.vector.tensor_mul(oT[h][:, sjs[jj]],
                                                 pos[jj][:], rec[:])
                        # stage this head's token segments and re-shard at
                        # once: the AllToAll overlaps the remaining attention
                        for jj in range(2):
                            j = half * 2 + jj
                            nc.sync.dma_start(
                                agi[half][h][jj * 4:(jj + 1) * 4, :, :]
                                .rearrange("s p t -> p s t"),
                                oT[h][:, j * 512:(j + 1) * 512]
                                .rearrange("p (s t) -> p s t", t=128))
                        if ag_mode != 'nocoll':
                            nc.gpsimd.collective_compute(
                                "AllToAll", mybir.AluOpType.bypass,
                                replica_groups=rg,
                                ins=[agi[half][h][:].opt()],
                                outs=[ato[half][h][:].opt()])
                        else:
                            nc.gpsimd.dma_start(ato[half][h][:],
                                                agi[half][h][:])

            if DEBUG_DUMP == "oT":
                nc.sync.dma_start(dbg[:], oT[0][:])
            if stage >= 4:
                # token-sharded projection, after attention's PSUM pools
                # close: 4 psum banks accumulate all 4 column groups with the
                # och stationary reused; contraction split by head parity so
                # the even-kt half starts before the head-1 AllToAll lands
                psE = ph4.enter_context(
                    tc.tile_pool(name=P("psE"), bufs=1, space="PSUM"))
                for half in range(2):
                    och = aw.tile([128, KT, 128], BF16, name="och", bufs=1)
                    for h in range(HPC):
                        nc.sync.dma_start(
                            och[:, h::2, :],
                            ato[half][h][:].rearrange("c p t -> p c t"))
                    for g in range(8):
                        # wo streamed per 256-column group (it would not fit
                        # resident at bf16); loads hide under attention
                        wog = aw.tile([128, KT, 256], BF16, name="wog")
                        nc.sync.dma_start(wog[:],
                                          woT[:, :, g * 256:(g + 1) * 256])
                        pout = psE.tile([128, 256], F32, name="pout", bufs=2)
                        for par in range(2):
                            for tt in range(KT // 2):
                                t = 2 * tt + par
                                nc.tensor.matmul(
                                    pout[:], och[:, t, :], wog[:, t, :],
                                    start=(par == 0 and tt == 0),
                                    stop=(par == 1 and tt == KT // 2 - 1))
                        ot = aw.tile([128, 256], F32, name="ot")
                        nc.vector.tensor_add(ot[:], pout[:],
                                             bo_bc[:, g * 256:(g + 1) * 256])
                        nc.sync.dma_start(
                            outTok[half, :, g * 256:(g + 1) * 256], ot[:])

    with tile.TileContext(nc) as tc:
        for rep in range(repeat):
            with ExitStack() as top:
                emit(tc, top, rep)

    nc.compile()
    return nc


def _prep_inputs(hidden_states, freqs_cos, freqs_sin, wq, bq, wk, bk, wv, bv,
                 norm_q_w, norm_k_w, wo, bo):
    """Host-side shard + layout prep. Returns in_maps for 8 cores."""
    f32 = np.float32
    hid = np.ascontiguousarray(np.asarray(hidden_states)[0].T, dtype=f32)
    hidT = np.zeros((DIM, SP), dtype=f32)
    hidT[:, :S] = hid
    import ml_dtypes
    bf16 = ml_dtypes.bfloat16
    fp8 = ml_dtypes.float8_e4m3fn
    # pre-tile to [chunk j, partition p, ktile t, col c]: d = t*128+p, s = j*512+c
    hidT = np.ascontiguousarray(
        hidT.reshape(KT, 128, SP // 512, 512).transpose(2, 1, 0, 3)).astype(bf16)

    def tile_w(wT, dt=bf16, scale=1.0):   # [DIM, DC] -> [128, KT, DC]
        return np.ascontiguousarray(
            (wT * scale).reshape(KT, 128, DC).transpose(1, 0, 2)).astype(dt)

    # RoPE tables: c_j[s] = cos[0,s,0,2j], s_j[s] = sin[0,s,0,2j+1]; stack [t;t]
    c = np.asarray(freqs_cos)[0, :, 0, 0::2].astype(f32).T          # [64, S]
    s = np.asarray(freqs_sin)[0, :, 0, 1::2].astype(f32).T          # [64, S]
    cosT = np.zeros((DH, SP), dtype=f32)
    sinT = np.zeros((DH, SP), dtype=f32)
    cosT[0:64, :S] = c
    cosT[64:128, :S] = c
    sinT[0:64, :S] = -s
    sinT[64:128, :S] = s

    perm = np.concatenate([np.arange(0, DH, 2), np.arange(1, DH, 2)])
    wq = np.asarray(wq)
    wk = np.asarray(wk)
    wv = np.asarray(wv)
    wo = np.asarray(wo)
    bqv = np.asarray(bq)
    bkv = np.asarray(bk)
    bvv = np.asarray(bv)
    bov = np.asarray(bo)
    nq = np.asarray(norm_q_w)
    nk = np.asarray(norm_k_w)

    # full wo, transposed+tiled: woT[p, t, j] = wo[j, t*128+p]; replicated
    woT_full = np.ascontiguousarray(
        wo.astype(f32).T.reshape(KT, 128, DIM).transpose(1, 0, 2)).astype(bf16)
    bo_full = bov.astype(f32).reshape(1, DIM)

    in_maps = []
    for core in range(N_CORES):
        rows = slice(core * DC, (core + 1) * DC)

        def permuted(mat_rows):                                     # [DC, DIM]
            blocks = [mat_rows[h * DH:(h + 1) * DH][perm] for h in range(HPC)]
            return np.concatenate(blocks, axis=0)

        def permuted_vec(vec_rows):                                 # [HPC, DH]
            blocks = [vec_rows[h * DH:(h + 1) * DH][perm] for h in range(HPC)]
            return np.stack(blocks, axis=0)

        wq_c = permuted(wq[rows].astype(f32))
        wk_c = permuted(wk[rows].astype(f32))
        in_maps.append({
            "hidT": hidT,
            "wqT": tile_w(np.ascontiguousarray(wq_c.T)),
            "wkT": tile_w(np.ascontiguousarray(wk_c.T)),
            "wvT": tile_w(np.ascontiguousarray(wv[rows].astype(f32).T)),
            "woT": woT_full,
            "cosT": cosT,
            "sinT": sinT,
            "bq": permuted_vec(bqv[rows].astype(f32)),
            "bk": permuted_vec(bkv[rows].astype(f32)),
            "bv": bvv[rows].astype(f32).reshape(1, DC),
            "bo": bo_full,
            "nwq": permuted_vec(nq[rows].astype(f32)),
            "nwk": permuted_vec(nk[rows].astype(f32)),
            "ones8": np.ones((128, 128), dtype=bf16),
        })
    return in_maps


def _assemble(results):
    """results[core]["outTok"] is [2, 128, DIM]: token blocks
    [m*128,(m+1)*128) and [1024+m*128, 1024+(m+1)*128)."""
    out = np.empty((1, S, DIM), dtype=np.float32)
    for m in range(N_CORES):
        blk = results[m]["outTok"]
        out[0, m * 128:(m + 1) * 128, :] = blk[0]
        lo = 1024 + m * 128
        n = min(128, S - lo)
        if n > 0:
            out[0, lo:lo + n, :] = blk[1][:n]
    return out


def kernel(**inputs):
    global _COMPILED
    if _COMPILED is None:
        _COMPILED = _build()
    nc = _COMPILED
    in_maps = _prep_inputs(**inputs)
    res = run_bass_kernel_spmd(nc, in_maps, core_ids=list(range(N_CORES)))
    return _assemble(res.results)

